# revision 45
# baseline (speedup 1.0000x reference)
"""Multi-head attention (B=2, N=2048, C=1024, H=16, D=64) on 8 TRN2 NeuronCores.

Sharding: tensor-parallel over heads. Core i owns heads (2i, 2i+1):
  - qkv weight columns for those heads (Q^T/K^T/V^T computed on device),
  - attention for 4 (batch, head) instances,
  - partial projection y_i = O_i @ W_proj[:, cols_i].T  (row-parallel proj).
Host gathers: y = sum_i y_i + b_proj.

Per-core pipeline (per batch):
  qkv:   Q^T,K^T [128(d,2 heads),2048] and V^T -> PE-transpose -> V_aug [n,130]
         (V columns + a ones column per head, so PV also yields softmax row-sums)
  attn:  per 512-query tile, loop over 16 key tiles:
         S^T[k,h,q] = K^T.T @ Q^T (bf16 in, f32 PSUM, the two heads run as
         packed row-group tiles), exp on ScalarE -> bf16,
         O~aug^T[65,q] += V_aug.T @ P~ (PSUM accumulation, row 64 = sum exp)
  norm:  rowsum row -> partition 0 (DMA) -> GpSimd broadcast ->
         fast reciprocal -> DVE mult (heads stacked for a K=128 projection)
  proj:  y[q,o] = sum_h O_norm^T_h.T @ WpT_h (PSUM accumulation over heads)

Batch 1's qkv work is emitted interleaved with batch 0's attention so the
TensorEngine stays dense (HAM stays at full clock) while ScalarE runs exp.
Matmul operands are bf16; softmax statistics, PSUM accumulation and the
final output stay float32.
"""
import sys
import types

import numpy as np

B = 2
N = 2048
C = 1024
H = 16
D = 64
SCALE = D ** -0.5
NCORES = 8
HPC = H // NCORES  # heads per core = 2
BN = B * N


def _install_ntff_shim():
    """The image's antenv lacks axon_hooks; provide it so trace=True works."""
    if "antenv.axon_hooks" in sys.modules:
        return
    mod = types.ModuleType("antenv.axon_hooks")
    mod._HOOK = None
    mod.set_axon_ntff_profile_hook = lambda h: setattr(mod, "_HOOK", h)
    mod.get_axon_ntff_profile_hook = lambda: mod._HOOK
    sys.modules["antenv.axon_hooks"] = mod
    if "/root/.axon_site" not in sys.path:
        sys.path.insert(0, "/root/.axon_site")
    try:
        from trn_agent_boot.trn_boot import _ntff_profile_via_ctypes

        mod.set_axon_ntff_profile_hook(
            _ntff_profile_via_ctypes("/opt/axon/libaxon_pjrt.so")
        )
    except Exception:
        pass


_install_ntff_shim()

import ml_dtypes  # noqa: E402

import concourse.bass as bass  # noqa: E402
import concourse.tile as tile  # noqa: E402
from concourse import bacc, mybir  # noqa: E402
from concourse.bass_utils import run_bass_kernel_spmd  # noqa: E402
from concourse.masks import make_identity  # noqa: E402

F32 = mybir.dt.float32
BF16 = mybir.dt.bfloat16
EXP = mybir.ActivationFunctionType.Exp
BFNP = ml_dtypes.bfloat16

NT = N // 512          # 512-token tiles per batch (4)
NKT = N // 128         # 128-token key tiles per batch (16)
CO = C // 128          # contraction chunks (8)

_NC_CACHE = {}


def build_nc():
    nc = bacc.Bacc(None, target_bir_lowering=False)

    xT_ext = nc.declare_dram_parameter("xT", [B, NT, C, 512], BF16, isOutput=False)
    wqT_ext = nc.declare_dram_parameter("wqT", [128, CO, 128], BF16, isOutput=False)
    wkT_ext = nc.declare_dram_parameter("wkT", [128, CO, 128], BF16, isOutput=False)
    wvT_ext = nc.declare_dram_parameter("wvT", [128, CO, 128], BF16, isOutput=False)
    wpT_ext = nc.declare_dram_parameter("wpT", [HPC, D, C], BF16, isOutput=False)
    out_ext = nc.declare_dram_parameter("out", [BN // 128, 2, 128, 512], BF16, isOutput=True)

    with tile.TileContext(nc) as tc:
        with (
            tc.tile_pool(name="consts", bufs=1) as consts,
            tc.tile_pool(name="weights", bufs=1) as weights,
            tc.tile_pool(name="xpool", bufs=8 * NT) as xpool,
            tc.tile_pool(name="qkvp", bufs=2 * NT) as qkvp,
            tc.tile_pool(name="work", bufs=8) as work,
            tc.tile_pool(name="small", bufs=3) as small,
            tc.tile_pool(name="onorm", bufs=6) as onormp,
            tc.tile_pool(name="ypool", bufs=4) as ypool,
            tc.tile_pool(name="ps_s", bufs=2, space="PSUM") as ps_s,
            tc.tile_pool(name="ps_o", bufs=3, space="PSUM") as ps_o,
            tc.tile_pool(name="ps_fill", bufs=1, space="PSUM") as ps_fill,
        ):
            wrm = consts.tile([128, 512], BF16)
            nc.vector.memset(wrm[:], 0.0)
            wps = ps_fill.tile([128, 512], F32, tag="fill", name="warm_ps")
            for _ in range(10):
                nc.tensor.matmul(
                    wps[:], lhsT=wrm[:, 0:128], rhs=wrm[:], start=True, stop=True
                )
            ident = consts.tile([128, 128], BF16)
            make_identity(nc, ident[:])
            onesb = consts.tile([128, 64], BF16)
            nc.vector.memset(onesb[:], 1.0)

            wq = weights.tile([128, CO, 128], BF16)
            wk = weights.tile([128, CO, 128], BF16)
            wv = weights.tile([128, CO, 128], BF16)
            wp = weights.tile([128, C], BF16)
            wp2 = weights.tile([D, C], BF16)
            nc.sync.dma_start(wq[:], wqT_ext[:])

            from collections import deque

            filler = deque()

            def load_x_tile(b, t):
                """x^T columns for 512 tokens: 8 chunk tiles of [128, 512]."""
                xs = []
                for co in range(CO):
                    xc = xpool.tile([128, 512], BF16, tag="xchunk", name="xc")
                    nc.sync.dma_start(
                        xc[:], xT_ext[b, t, co * 128:(co + 1) * 128, :]
                    )
                    xs.append(xc)
                return xs

            def qkv_chunk_units(b, t, xs, store):
                """One 512-token qkv tile -> QTc/KTc/VAc chunk tiles.

                Returns a list of single-PE-instruction closures."""
                units = []
                QTc = qkvp.tile([128, 512], BF16, tag="qt", name="qtc")
                KTc = qkvp.tile([128, 512], BF16, tag="kt", name="ktc")
                VAc = qkvp.tile([128, 4, 2 * (D + 1)], BF16, tag="vaug", name="vac")
                nc.vector.tensor_copy(VAc[:, :, D], onesb[:, 0:4])
                nc.vector.tensor_copy(VAc[:, :, 2 * D + 1], onesb[:, 0:4])
                store[t] = (QTc, KTc, VAc)

                def chain(w, writer):
                    hold = {}

                    def unit(co):
                        def f():
                            if co == 0:
                                hold["ps"] = ps_fill.tile(
                                    [128, 512], F32, tag="fill", name="fill_ps"
                                )
                            nc.tensor.matmul(
                                hold["ps"][:], lhsT=w[:, co, :], rhs=xs[co][:],
                                start=(co == 0), stop=(co == CO - 1),
                            )
                            if co == CO - 1:
                                writer(hold["ps"])
                        return f

                    return [unit(co) for co in range(CO)]

                vhold = {}

                def v_writer(ps):
                    vhold["vt"] = work.tile(
                        [128, 512], BF16, tag="vt", name="vt_stage"
                    )
                    nc.vector.tensor_copy(vhold["vt"][:], ps[:])

                units += chain(wq, lambda ps: nc.vector.tensor_copy(QTc[:], ps[:]))
                units += chain(wk, lambda ps: nc.vector.tensor_copy(KTc[:], ps[:]))
                units += chain(wv, v_writer)

                def transpose_unit(j):
                    def f():
                        pst = ps_fill.tile(
                            [128, 128], BF16, tag="fill", name="fill_t"
                        )
                        nc.tensor.transpose(
                            pst[:], vhold["vt"][:, j * 128:(j + 1) * 128], ident[:]
                        )
                        nc.vector.tensor_copy(
                            VAc[:, j, :]
                            .rearrange("p (h e) -> p h e", e=D + 1)[:, :, 0:D],
                            pst.rearrange("p (h d) -> p h d", d=D),
                        )
                    return f

                units += [transpose_unit(j) for j in range(4)]
                return units

            def queue_proj(on_stk, b, qt, onh1=None):
                def unit(sq, ot):
                    def f():
                        sqs = slice(sq * 128, (sq + 1) * 128)
                        qb = (b * N + qt * 512 + sq * 128) // 128
                        os_ = slice(ot * 512, (ot + 1) * 512)
                        yps = ps_fill.tile(
                            [128, 512], F32, tag="fill", name="fill_y"
                        )
                        if onh1 is None:
                            nc.tensor.matmul(
                                yps[:], lhsT=on_stk[:, sqs], rhs=wp[:, os_],
                                start=True, stop=True,
                            )
                        else:
                            # tail fast path: both heads from base-0 tiles,
                            # no partition-shift DMA on the critical path
                            nc.tensor.matmul(
                                yps[:], lhsT=on_stk[0:D, sqs], rhs=wp[0:D, os_],
                                start=True, stop=False,
                            )
                            nc.tensor.matmul(
                                yps[:], lhsT=onh1[:, sqs], rhs=wp2[:, os_],
                                start=False, stop=True,
                            )
                        ysb = ypool.tile([128, 512], BF16, tag="y", name="ysb")
                        nc.vector.tensor_copy(ysb[:], yps[:])
                        nc.sync.dma_start(out_ext[qb, ot], ysb[:])
                    return f

                for sq in range(4):
                    for ot in range(2):
                        filler.append(unit(sq, ot))

            PVLAG = 6

            def norm_and_queue_proj(O0, O1, b, qt, last=False):
                on_stk = onormp.tile([128, 512], BF16, tag="onstk", name="onstk")
                onh1 = None
                for h, O in ((0, O0), (1, O1)):
                    rec = small.tile([128, 512], F32, tag="rec", name="rec")
                    nc.vector.tensor_copy(rec[D:D + 1, :], O[D:D + 1, :])
                    rec0 = small.tile([128, 512], F32, tag="rec0", name="rec0")
                    nc.sync.dma_start(rec0[0:1, :], rec[D:D + 1, :])
                    sums = small.tile([D, 512], F32, tag="sums", name="sums")
                    nc.gpsimd.partition_broadcast(sums[:], rec0[0:1, :], channels=D)
                    bcs = small.tile([D, 512], F32, tag="bcs", name="bcs")
                    nc.vector.reciprocal_approx_fast(bcs[:], sums[:])
                    if h == 0:
                        nc.vector.tensor_mul(on_stk[0:D, :], O[0:D, :], bcs[:])
                    else:
                        onh = onormp.tile([D, 512], BF16, tag="onh", name="onh")
                        nc.vector.tensor_mul(onh[:], O[0:D, :], bcs[:])
                        if last:
                            onh1 = onh
                        else:
                            nc.sync.dma_start(on_stk[D:2 * D, :], onh[:])
                queue_proj(on_stk, b, qt, onh1)

            # ---- schedule: one global software pipeline ----
            store0, store1 = {}, {}
            x0 = [load_x_tile(0, 0)]
            nc.sync.dma_start(wk[:], wkT_ext[:])
            nc.sync.dma_start(wv[:], wvT_ext[:])
            x0 += [load_x_tile(0, t) for t in range(1, NT)]
            nc.sync.dma_start(wp[:], wpT_ext.rearrange("h p o -> (h p) o"))
            nc.sync.dma_start(wp2[:], wpT_ext[1])
            u0 = [qkv_chunk_units(0, t, x0[t], store0) for t in range(NT)]
            for u in u0[0][:16]:     # Q + K chains: needed by the first S
                u()
            x1 = [load_x_tile(1, t) for t in range(NT)]
            u1 = [qkv_chunk_units(1, t, x1[t], store1) for t in range(NT)]

            # qkv work in full-chain groups (atomic: they share one PSUM slot)
            qkv_groups = deque()

            def add_tile_groups(dl, ut):
                for c in range(3):
                    qkv_groups.append((dl, ut[8 * c:8 * c + 8], False))
                qkv_groups.append((dl, ut[24:28], False))

            qkv_groups.append((0, u0[0][16:24], False))
            qkv_groups.append((1, u0[0][24:28], False))
            for t in range(1, NT):
                add_tile_groups(4 * t, u0[t])
            for t in range(NT):
                add_tile_groups(64 + 4 * t, u1[t])
            n_groups_total = len(qkv_groups)
            pulled = [0]

            chain_open = [False]

            def pace_qkv(g):
                if qkv_groups and (
                    chain_open[0]
                    or qkv_groups[0][0] <= g + 4
                ):
                    dl, units, opens = qkv_groups.popleft()
                    for u in units:
                        u()
                    chain_open[0] = opens
                    pulled[0] += 1
                    return True
                return False

            stores = {0: store0, 1: store1}
            qts = [(0, q) for q in range(NT)] + [(1, q) for q in range(NT)]
            NQ = len(qts)
            Otiles = {}
            Ps = {}
            for g in range(NQ * NKT + PVLAG):
                # retire: PV pair for iteration g - PVLAG
                pv = g - PVLAG
                if pv >= 0:
                    bp, qtp = qts[pv // NKT]
                    ktp = pv % NKT
                    if ktp == 0:
                        Otiles[pv // NKT] = (
                            ps_o.tile([D + 1, 512], F32, tag="oacc", name="o0"),
                            ps_o.tile([D + 1, 512], F32, tag="oacc", name="o1"),
                        )
                    O0, O1 = Otiles[pv // NKT]
                    VAc = stores[bp][ktp // 4][2]
                    j = ktp % 4
                    P = Ps.pop(pv)
                    nc.tensor.matmul(
                        O0[:], lhsT=VAc[:, j, 0:D + 1], rhs=P[:, 0, :],
                        start=(ktp == 0), stop=(ktp == NKT - 1),
                    )
                    nc.tensor.matmul(
                        O1[:], lhsT=VAc[:, j, D + 1:2 * (D + 1)], rhs=P[:, 1, :],
                        start=(ktp == 0), stop=(ktp == NKT - 1),
                    )
                    if ktp == NKT - 1:
                        norm_and_queue_proj(
                            O0, O1, bp, qtp, last=(pv // NKT == NQ - 1)
                        )
                        del Otiles[pv // NKT]
                # issue: S pair + exp for iteration g
                chain_pulled = False
                if g < NQ * NKT:
                    b, qt = qts[g // NKT]
                    kt = g % NKT
                    chain_pulled = pace_qkv(g)
                    QTc = stores[b][qt][0]
                    KTc = stores[b][kt // 4][1]
                    ks = slice((kt % 4) * 128, (kt % 4 + 1) * 128)
                    S = ps_s.tile([128, 2, 512], F32, tag="s", name="s")
                    nc.tensor.matmul(
                        S[:, 0, :], lhsT=KTc[0:D, ks], rhs=QTc[0:D, :],
                        start=True, stop=True, tile_position=(0, 0),
                    )
                    nc.tensor.matmul(
                        S[:, 1, :], lhsT=KTc[D:2 * D, ks], rhs=QTc[D:2 * D, :],
                        start=True, stop=True, tile_position=(64, 0),
                    )
                    P = work.tile([128, 2, 512], BF16, tag="p", name="p")
                    nc.scalar.activation(P[:], S[:], EXP)
                    Ps[g] = P
                # drip proj fillers (not on chain iterations: proj shares the
                # single PSUM fill slot with open chains)
                if not chain_pulled:
                    for _ in range(2 if g >= 64 else 1):
                        if filler:
                            filler.popleft()()
            while filler or qkv_groups:
                if qkv_groups:
                    for u in qkv_groups.popleft()[1]:
                        u()
                elif filler:
                    filler.popleft()()
    nc.finalize()
    return nc


def _host_prep(x, W_qkv, W_proj):
    xT = np.ascontiguousarray(
        x.reshape(B, N // 512, 512, C).transpose(0, 1, 3, 2)
    ).astype(BFNP)
    in_maps = []
    for i in range(NCORES):
        hs = [HPC * i + j for j in range(HPC)]
        wq = np.concatenate([W_qkv[h * D:(h + 1) * D, :] for h in hs], 0)
        wk = np.concatenate([W_qkv[C + h * D:C + (h + 1) * D, :] for h in hs], 0)
        wv = np.concatenate([W_qkv[2 * C + h * D:2 * C + (h + 1) * D, :] for h in hs], 0)
        def chunkw(wT):
            return np.ascontiguousarray(
                wT.reshape(CO, 128, 128).transpose(1, 0, 2)
            ).astype(BFNP)

        wqT = chunkw((wq * SCALE).T)
        wkT = chunkw(wk.T)
        wvT = chunkw(wv.T)
        wpT = np.stack(
            [np.ascontiguousarray(W_proj[:, h * D:(h + 1) * D].T) for h in hs], 0
        ).astype(BFNP)
        in_maps.append({"xT": xT, "wqT": wqT, "wkT": wkT, "wvT": wvT, "wpT": wpT})
    return in_maps


def run(x, W_qkv, W_proj, b_proj, trace=False):
    if "nc" not in _NC_CACHE:
        _NC_CACHE["nc"] = build_nc()
    nc = _NC_CACHE["nc"]
    in_maps = _host_prep(np.asarray(x), np.asarray(W_qkv), np.asarray(W_proj))
    res = run_bass_kernel_spmd(
        nc, in_maps, core_ids=list(range(NCORES)), trace=trace
    )
    acc = np.zeros((BN, C), np.float64)
    for i in range(NCORES):
        o = res.results[i]["out"].astype(np.float64)   # [BN//128, 2, 128, 512]
        acc += o.transpose(0, 2, 1, 3).reshape(BN, C)
    y = (acc + np.asarray(b_proj).astype(np.float64)).astype(np.float32)
    return y.reshape(B, N, C), res


def kernel(x, W_qkv, W_proj, b_proj):
    y, _ = run(x, W_qkv, W_proj, b_proj, trace=False)
    return y


# revision 47
# speedup vs baseline: 1.0268x; 1.0268x over previous
"""Multi-head attention (B=2, N=2048, C=1024, H=16, D=64) on 8 TRN2 NeuronCores.

Sharding: tensor-parallel over heads. Core i owns heads (2i, 2i+1):
  - qkv weight columns for those heads (Q^T/K^T/V^T computed on device),
  - attention for 4 (batch, head) instances,
  - partial projection y_i = O_i @ W_proj[:, cols_i].T  (row-parallel proj).
Host gathers: y = sum_i y_i + b_proj.

Per-core pipeline (per batch):
  qkv:   Q^T,K^T [128(d,2 heads),2048] and V^T -> PE-transpose -> V_aug [n,130]
         (V columns + a ones column per head, so PV also yields softmax row-sums)
  attn:  per 512-query tile, loop over 16 key tiles:
         S^T[k,h,q] = K^T.T @ Q^T (bf16 in, f32 PSUM, the two heads run as
         packed row-group tiles), exp on ScalarE -> bf16,
         O~aug^T[65,q] += V_aug.T @ P~ (PSUM accumulation, row 64 = sum exp)
  norm:  rowsum row -> partition 0 (DMA) -> GpSimd broadcast ->
         fast reciprocal -> DVE mult (heads stacked for a K=128 projection)
  proj:  y[q,o] = sum_h O_norm^T_h.T @ WpT_h (PSUM accumulation over heads)

Batch 1's qkv work is emitted interleaved with batch 0's attention so the
TensorEngine stays dense (HAM stays at full clock) while ScalarE runs exp.
Matmul operands are bf16; softmax statistics, PSUM accumulation and the
final output stay float32.
"""
import sys
import types

import numpy as np

B = 2
N = 2048
C = 1024
H = 16
D = 64
SCALE = D ** -0.5
NCORES = 8
HPC = H // NCORES  # heads per core = 2
BN = B * N


def _install_ntff_shim():
    """The image's antenv lacks axon_hooks; provide it so trace=True works."""
    if "antenv.axon_hooks" in sys.modules:
        return
    mod = types.ModuleType("antenv.axon_hooks")
    mod._HOOK = None
    mod.set_axon_ntff_profile_hook = lambda h: setattr(mod, "_HOOK", h)
    mod.get_axon_ntff_profile_hook = lambda: mod._HOOK
    sys.modules["antenv.axon_hooks"] = mod
    if "/root/.axon_site" not in sys.path:
        sys.path.insert(0, "/root/.axon_site")
    try:
        from trn_agent_boot.trn_boot import _ntff_profile_via_ctypes

        mod.set_axon_ntff_profile_hook(
            _ntff_profile_via_ctypes("/opt/axon/libaxon_pjrt.so")
        )
    except Exception:
        pass


_install_ntff_shim()

import ml_dtypes  # noqa: E402

import concourse.bass as bass  # noqa: E402
import concourse.tile as tile  # noqa: E402
from concourse import bacc, mybir  # noqa: E402
from concourse.bass_utils import run_bass_kernel_spmd  # noqa: E402
from concourse.masks import make_identity  # noqa: E402

F32 = mybir.dt.float32
BF16 = mybir.dt.bfloat16
EXP = mybir.ActivationFunctionType.Exp
BFNP = ml_dtypes.bfloat16

NT = N // 512          # 512-token tiles per batch (4)
NKT = N // 128         # 128-token key tiles per batch (16)
CO = C // 128          # contraction chunks (8)

_NC_CACHE = {}


def build_nc():
    nc = bacc.Bacc(None, target_bir_lowering=False)

    xT_ext = nc.declare_dram_parameter("xT", [B, NT, C, 512], BF16, isOutput=False)
    wqT_ext = nc.declare_dram_parameter("wqT", [CO, 128, 128], BF16, isOutput=False)
    wkT_ext = nc.declare_dram_parameter("wkT", [CO, 128, 128], BF16, isOutput=False)
    wvT_ext = nc.declare_dram_parameter("wvT", [CO, 128, 128], BF16, isOutput=False)
    wpT_ext = nc.declare_dram_parameter("wpT", [HPC, D, C], BF16, isOutput=False)
    out_ext = nc.declare_dram_parameter("out", [BN // 128, 2, 128, 512], BF16, isOutput=True)

    with tile.TileContext(nc) as tc:
        with (
            tc.tile_pool(name="consts", bufs=1) as consts,
            tc.tile_pool(name="weights", bufs=1) as weights,
            tc.tile_pool(name="xpool", bufs=8 * NT) as xpool,
            tc.tile_pool(name="qkvp", bufs=2 * NT) as qkvp,
            tc.tile_pool(name="work", bufs=8) as work,
            tc.tile_pool(name="small", bufs=3) as small,
            tc.tile_pool(name="onorm", bufs=6) as onormp,
            tc.tile_pool(name="ypool", bufs=8) as ypool,
            tc.tile_pool(name="ps_s", bufs=2, space="PSUM") as ps_s,
            tc.tile_pool(name="ps_o", bufs=3, space="PSUM") as ps_o,
            tc.tile_pool(name="ps_fill", bufs=1, space="PSUM") as ps_fill,
        ):
            wrm = consts.tile([128, 512], BF16)
            nc.vector.memset(wrm[:], 0.0)
            wps = ps_fill.tile([128, 512], F32, tag="fill", name="warm_ps")
            for _ in range(10):
                nc.tensor.matmul(
                    wps[:], lhsT=wrm[:, 0:128], rhs=wrm[:], start=True, stop=True
                )
            ident = consts.tile([128, 128], BF16)
            make_identity(nc, ident[:])
            onesb = consts.tile([128, 64], BF16)
            nc.vector.memset(onesb[:], 1.0)

            def wchunks(ext, nm):
                ts_ = []
                for co in range(CO):
                    t = weights.tile([128, 128], BF16, name=f"{nm}{co}")
                    nc.sync.dma_start(t[:], ext[co])
                    ts_.append(t)
                return ts_

            wq = wchunks(wqT_ext, "wq")
            wp = weights.tile([128, C], BF16)
            wp2 = weights.tile([D, C], BF16)

            from collections import deque

            filler = deque()

            def load_x_tile(b, t):
                """x^T columns for 512 tokens: 8 chunk tiles of [128, 512]."""
                xs = []
                for co in range(CO):
                    xc = xpool.tile([128, 512], BF16, tag="xchunk", name="xc")
                    nc.sync.dma_start(
                        xc[:], xT_ext[b, t, co * 128:(co + 1) * 128, :]
                    )
                    xs.append(xc)
                return xs

            def qkv_chunk_units(b, t, xs, store):
                """One 512-token qkv tile -> QTc/KTc/VAc chunk tiles.

                Returns a list of single-PE-instruction closures."""
                units = []
                QTc = qkvp.tile([128, 512], BF16, tag="qt", name="qtc")
                KTc = qkvp.tile([128, 512], BF16, tag="kt", name="ktc")
                VAc = qkvp.tile([128, 4, 2 * (D + 1)], BF16, tag="vaug", name="vac")
                nc.vector.tensor_copy(VAc[:, :, D], onesb[:, 0:4])
                nc.vector.tensor_copy(VAc[:, :, 2 * D + 1], onesb[:, 0:4])
                store[t] = (QTc, KTc, VAc)

                def chain(w, writer):
                    hold = {}

                    def unit(co):
                        def f():
                            if co == 0:
                                hold["ps"] = ps_fill.tile(
                                    [128, 512], F32, tag="fill", name="fill_ps"
                                )
                            nc.tensor.matmul(
                                hold["ps"][:], lhsT=w[co][:], rhs=xs[co][:],
                                start=(co == 0), stop=(co == CO - 1),
                            )
                            if co == CO - 1:
                                writer(hold["ps"])
                        return f

                    return [unit(co) for co in range(CO)]

                vhold = {}

                def v_writer(ps):
                    vhold["vt"] = work.tile(
                        [128, 512], BF16, tag="vt", name="vt_stage"
                    )
                    nc.vector.tensor_copy(vhold["vt"][:], ps[:])

                units += chain(wq, lambda ps: nc.vector.tensor_copy(QTc[:], ps[:]))
                units += chain(wk, lambda ps: nc.vector.tensor_copy(KTc[:], ps[:]))
                units += chain(wv, v_writer)

                def transpose_unit(j):
                    def f():
                        pst = ps_fill.tile(
                            [128, 128], BF16, tag="fill", name="fill_t"
                        )
                        nc.tensor.transpose(
                            pst[:], vhold["vt"][:, j * 128:(j + 1) * 128], ident[:]
                        )
                        nc.vector.tensor_copy(
                            VAc[:, j, :]
                            .rearrange("p (h e) -> p h e", e=D + 1)[:, :, 0:D],
                            pst.rearrange("p (h d) -> p h d", d=D),
                        )
                    return f

                units += [transpose_unit(j) for j in range(4)]
                return units

            def queue_proj(on_stk, b, qt, onh1=None):
                def unit(sq, ot):
                    def f():
                        sqs = slice(sq * 128, (sq + 1) * 128)
                        qb = (b * N + qt * 512 + sq * 128) // 128
                        os_ = slice(ot * 512, (ot + 1) * 512)
                        yps = ps_fill.tile(
                            [128, 512], F32, tag="fill", name="fill_y"
                        )
                        if onh1 is None:
                            nc.tensor.matmul(
                                yps[:], lhsT=on_stk[:, sqs], rhs=wp[:, os_],
                                start=True, stop=True,
                            )
                        else:
                            # tail fast path: both heads from base-0 tiles,
                            # no partition-shift DMA on the critical path
                            nc.tensor.matmul(
                                yps[:], lhsT=on_stk[0:D, sqs], rhs=wp[0:D, os_],
                                start=True, stop=False,
                            )
                            nc.tensor.matmul(
                                yps[:], lhsT=onh1[:, sqs], rhs=wp2[:, os_],
                                start=False, stop=True,
                            )
                        ysb = ypool.tile([128, 512], BF16, tag="y", name="ysb")
                        nc.vector.tensor_copy(ysb[:], yps[:])
                        nc.sync.dma_start(out_ext[qb, ot], ysb[:])
                    return f

                for sq in range(4):
                    for ot in range(2):
                        filler.append(unit(sq, ot))

            PVLAG = 6

            def norm_and_queue_proj(O0, O1, b, qt, last=False):
                on_stk = onormp.tile([128, 512], BF16, tag="onstk", name="onstk")
                onh1 = None
                for h, O in ((0, O0), (1, O1)):
                    rec = small.tile([128, 512], F32, tag="rec", name="rec")
                    nc.vector.tensor_copy(rec[D:D + 1, :], O[D:D + 1, :])
                    rec0 = small.tile([128, 512], F32, tag="rec0", name="rec0")
                    nc.sync.dma_start(rec0[0:1, :], rec[D:D + 1, :])
                    sums = small.tile([D, 512], F32, tag="sums", name="sums")
                    nc.gpsimd.partition_broadcast(sums[:], rec0[0:1, :], channels=D)
                    bcs = small.tile([D, 512], F32, tag="bcs", name="bcs")
                    nc.vector.reciprocal_approx_fast(bcs[:], sums[:])
                    if h == 0:
                        nc.vector.tensor_mul(on_stk[0:D, :], O[0:D, :], bcs[:])
                    else:
                        onh = onormp.tile([D, 512], BF16, tag="onh", name="onh")
                        nc.vector.tensor_mul(onh[:], O[0:D, :], bcs[:])
                        if last:
                            onh1 = onh
                        else:
                            nc.sync.dma_start(on_stk[D:2 * D, :], onh[:])
                queue_proj(on_stk, b, qt, onh1)

            # ---- schedule: one global software pipeline ----
            store0, store1 = {}, {}
            x0 = [load_x_tile(0, 0)]
            wk = wchunks(wkT_ext, "wk")
            wv = wchunks(wvT_ext, "wv")
            x0 += [load_x_tile(0, t) for t in range(1, NT)]
            nc.sync.dma_start(wp[:], wpT_ext.rearrange("h p o -> (h p) o"))
            nc.sync.dma_start(wp2[:], wpT_ext[1])
            u0 = [qkv_chunk_units(0, t, x0[t], store0) for t in range(NT)]
            for u in u0[0][:16]:     # Q + K chains: needed by the first S
                u()
            x1 = [load_x_tile(1, t) for t in range(NT)]
            u1 = [qkv_chunk_units(1, t, x1[t], store1) for t in range(NT)]

            # qkv work in full-chain groups (atomic: they share one PSUM slot)
            qkv_groups = deque()

            def add_tile_groups(dl, ut):
                for c in range(3):
                    qkv_groups.append((dl, ut[8 * c:8 * c + 8], False))
                qkv_groups.append((dl, ut[24:28], False))

            qkv_groups.append((0, u0[0][16:24], False))
            qkv_groups.append((1, u0[0][24:28], False))
            for t in range(1, NT):
                add_tile_groups(4 * t, u0[t])
            for t in range(NT):
                add_tile_groups(52 + 4 * t, u1[t])
            n_groups_total = len(qkv_groups)
            pulled = [0]

            chain_open = [False]

            def pace_qkv(g):
                if qkv_groups and (
                    chain_open[0]
                    or qkv_groups[0][0] <= g + 3
                    or pulled[0] < (g * n_groups_total) // 70
                ):
                    dl, units, opens = qkv_groups.popleft()
                    for u in units:
                        u()
                    chain_open[0] = opens
                    pulled[0] += 1
                    return True
                return False

            stores = {0: store0, 1: store1}
            qts = [(0, q) for q in range(NT)] + [(1, q) for q in range(NT)]
            NQ = len(qts)
            Otiles = {}
            Ps = {}
            for g in range(NQ * NKT + PVLAG):
                # retire: PV pair for iteration g - PVLAG
                pv = g - PVLAG
                if pv >= 0:
                    bp, qtp = qts[pv // NKT]
                    ktp = pv % NKT
                    if ktp == 0:
                        Otiles[pv // NKT] = (
                            ps_o.tile([D + 1, 512], F32, tag="oacc", name="o0"),
                            ps_o.tile([D + 1, 512], F32, tag="oacc", name="o1"),
                        )
                    O0, O1 = Otiles[pv // NKT]
                    VAc = stores[bp][ktp // 4][2]
                    j = ktp % 4
                    P = Ps.pop(pv)
                    nc.tensor.matmul(
                        O0[:], lhsT=VAc[:, j, 0:D + 1], rhs=P[:, 0, :],
                        start=(ktp == 0), stop=(ktp == NKT - 1),
                    )
                    nc.tensor.matmul(
                        O1[:], lhsT=VAc[:, j, D + 1:2 * (D + 1)], rhs=P[:, 1, :],
                        start=(ktp == 0), stop=(ktp == NKT - 1),
                    )
                    if ktp == NKT - 1:
                        norm_and_queue_proj(
                            O0, O1, bp, qtp, last=(pv // NKT == NQ - 1)
                        )
                        del Otiles[pv // NKT]
                # issue: S pair + exp for iteration g
                chain_pulled = False
                if g < NQ * NKT:
                    b, qt = qts[g // NKT]
                    kt = g % NKT
                    chain_pulled = pace_qkv(g)
                    QTc = stores[b][qt][0]
                    KTc = stores[b][kt // 4][1]
                    ks = slice((kt % 4) * 128, (kt % 4 + 1) * 128)
                    S = ps_s.tile([128, 2, 512], F32, tag="s", name="s")
                    nc.tensor.matmul(
                        S[:, 0, :], lhsT=KTc[0:D, ks], rhs=QTc[0:D, :],
                        start=True, stop=True, tile_position=(0, 0),
                    )
                    nc.tensor.matmul(
                        S[:, 1, :], lhsT=KTc[D:2 * D, ks], rhs=QTc[D:2 * D, :],
                        start=True, stop=True, tile_position=(64, 0),
                    )
                    P = work.tile([128, 2, 512], BF16, tag="p", name="p")
                    nc.scalar.activation(P[:], S[:], EXP)
                    Ps[g] = P
                # drip proj fillers (not on chain iterations: proj shares the
                # single PSUM fill slot with open chains)
                if not chain_pulled:
                    for _ in range(2 if g >= 64 else 1):
                        if filler:
                            filler.popleft()()
            while filler or qkv_groups:
                if qkv_groups:
                    for u in qkv_groups.popleft()[1]:
                        u()
                elif filler:
                    filler.popleft()()
    nc.finalize()
    return nc


def _host_prep(x, W_qkv, W_proj):
    xT = np.ascontiguousarray(
        x.reshape(B, N // 512, 512, C).transpose(0, 1, 3, 2)
    ).astype(BFNP)
    in_maps = []
    for i in range(NCORES):
        hs = [HPC * i + j for j in range(HPC)]
        wq = np.concatenate([W_qkv[h * D:(h + 1) * D, :] for h in hs], 0)
        wk = np.concatenate([W_qkv[C + h * D:C + (h + 1) * D, :] for h in hs], 0)
        wv = np.concatenate([W_qkv[2 * C + h * D:2 * C + (h + 1) * D, :] for h in hs], 0)
        def chunkw(wT):
            return np.ascontiguousarray(wT.reshape(CO, 128, 128)).astype(BFNP)

        wqT = chunkw((wq * SCALE).T)
        wkT = chunkw(wk.T)
        wvT = chunkw(wv.T)
        wpT = np.stack(
            [np.ascontiguousarray(W_proj[:, h * D:(h + 1) * D].T) for h in hs], 0
        ).astype(BFNP)
        in_maps.append({"xT": xT, "wqT": wqT, "wkT": wkT, "wvT": wvT, "wpT": wpT})
    return in_maps


def run(x, W_qkv, W_proj, b_proj, trace=False):
    if "nc" not in _NC_CACHE:
        _NC_CACHE["nc"] = build_nc()
    nc = _NC_CACHE["nc"]
    in_maps = _host_prep(np.asarray(x), np.asarray(W_qkv), np.asarray(W_proj))
    res = run_bass_kernel_spmd(
        nc, in_maps, core_ids=list(range(NCORES)), trace=trace
    )
    acc = np.zeros((BN, C), np.float64)
    for i in range(NCORES):
        o = res.results[i]["out"].astype(np.float64)   # [BN//128, 2, 128, 512]
        acc += o.transpose(0, 2, 1, 3).reshape(BN, C)
    y = (acc + np.asarray(b_proj).astype(np.float64)).astype(np.float32)
    return y.reshape(B, N, C), res


def kernel(x, W_qkv, W_proj, b_proj):
    y, _ = run(x, W_qkv, W_proj, b_proj, trace=False)
    return y


# revision 49
# speedup vs baseline: 1.0580x; 1.0304x over previous
"""Multi-head attention (B=2, N=2048, C=1024, H=16, D=64) on 8 TRN2 NeuronCores.

Sharding: tensor-parallel over heads. Core i owns heads (2i, 2i+1):
  - qkv weight columns for those heads (Q^T/K^T/V^T computed on device),
  - attention for 4 (batch, head) instances,
  - partial projection y_i = O_i @ W_proj[:, cols_i].T  (row-parallel proj).
Host gathers: y = sum_i y_i + b_proj.

Per-core pipeline (per batch):
  qkv:   Q^T,K^T [128(d,2 heads),2048] and V^T -> PE-transpose -> V_aug [n,130]
         (V columns + a ones column per head, so PV also yields softmax row-sums)
  attn:  per 512-query tile, loop over 16 key tiles:
         S^T[k,h,q] = K^T.T @ Q^T (bf16 in, f32 PSUM, the two heads run as
         packed row-group tiles), exp on ScalarE -> bf16,
         O~aug^T[65,q] += V_aug.T @ P~ (PSUM accumulation, row 64 = sum exp)
  norm:  rowsum row -> partition 0 (DMA) -> GpSimd broadcast ->
         fast reciprocal -> DVE mult (heads stacked for a K=128 projection)
  proj:  y[q,o] = sum_h O_norm^T_h.T @ WpT_h (PSUM accumulation over heads)

Batch 1's qkv work is emitted interleaved with batch 0's attention so the
TensorEngine stays dense (HAM stays at full clock) while ScalarE runs exp.
Matmul operands are bf16; softmax statistics, PSUM accumulation and the
final output stay float32.
"""
import sys
import types

import numpy as np

B = 2
N = 2048
C = 1024
H = 16
D = 64
SCALE = D ** -0.5
NCORES = 8
HPC = H // NCORES  # heads per core = 2
BN = B * N


def _install_ntff_shim():
    """The image's antenv lacks axon_hooks; provide it so trace=True works."""
    if "antenv.axon_hooks" in sys.modules:
        return
    mod = types.ModuleType("antenv.axon_hooks")
    mod._HOOK = None
    mod.set_axon_ntff_profile_hook = lambda h: setattr(mod, "_HOOK", h)
    mod.get_axon_ntff_profile_hook = lambda: mod._HOOK
    sys.modules["antenv.axon_hooks"] = mod
    if "/root/.axon_site" not in sys.path:
        sys.path.insert(0, "/root/.axon_site")
    try:
        from trn_agent_boot.trn_boot import _ntff_profile_via_ctypes

        mod.set_axon_ntff_profile_hook(
            _ntff_profile_via_ctypes("/opt/axon/libaxon_pjrt.so")
        )
    except Exception:
        pass


_install_ntff_shim()

import ml_dtypes  # noqa: E402

import concourse.bass as bass  # noqa: E402
import concourse.tile as tile  # noqa: E402
from concourse import bacc, mybir  # noqa: E402
from concourse.bass_utils import run_bass_kernel_spmd  # noqa: E402
from concourse.masks import make_identity  # noqa: E402

F32 = mybir.dt.float32
BF16 = mybir.dt.bfloat16
EXP = mybir.ActivationFunctionType.Exp
BFNP = ml_dtypes.bfloat16

NT = N // 512          # 512-token tiles per batch (4)
NKT = N // 128         # 128-token key tiles per batch (16)
CO = C // 128          # contraction chunks (8)

_NC_CACHE = {}


def build_nc():
    nc = bacc.Bacc(None, target_bir_lowering=False)

    xT_ext = nc.declare_dram_parameter("xT", [B, NT, C, 512], BF16, isOutput=False)
    wqT_ext = nc.declare_dram_parameter("wqT", [128, CO, 128], BF16, isOutput=False)
    wkT_ext = nc.declare_dram_parameter("wkT", [128, CO, 128], BF16, isOutput=False)
    wvT_ext = nc.declare_dram_parameter("wvT", [128, CO, 128], BF16, isOutput=False)
    wpT_ext = nc.declare_dram_parameter("wpT", [HPC, D, C], BF16, isOutput=False)
    out_ext = nc.declare_dram_parameter("out", [BN // 128, 2, 128, 512], BF16, isOutput=True)

    with tile.TileContext(nc) as tc:
        with (
            tc.tile_pool(name="consts", bufs=1) as consts,
            tc.tile_pool(name="weights", bufs=1) as weights,
            tc.tile_pool(name="xpool", bufs=8 * NT) as xpool,
            tc.tile_pool(name="qkvp", bufs=2 * NT) as qkvp,
            tc.tile_pool(name="work", bufs=8) as work,
            tc.tile_pool(name="small", bufs=3) as small,
            tc.tile_pool(name="onorm", bufs=6) as onormp,
            tc.tile_pool(name="ypool", bufs=4) as ypool,
            tc.tile_pool(name="ps_s", bufs=2, space="PSUM") as ps_s,
            tc.tile_pool(name="ps_o", bufs=3, space="PSUM") as ps_o,
            tc.tile_pool(name="ps_fill", bufs=1, space="PSUM") as ps_fill,
        ):
            wrm = consts.tile([128, 512], BF16)
            nc.vector.memset(wrm[:], 0.0)
            wps = ps_fill.tile([128, 512], F32, tag="fill", name="warm_ps")
            for _ in range(10):
                nc.tensor.matmul(
                    wps[:], lhsT=wrm[:, 0:128], rhs=wrm[:], start=True, stop=True
                )
            ident = consts.tile([128, 128], BF16)
            make_identity(nc, ident[:])
            onesb = consts.tile([128, 64], BF16)
            nc.vector.memset(onesb[:], 1.0)

            wq = weights.tile([128, CO, 128], BF16)
            wk = weights.tile([128, CO, 128], BF16)
            wv = weights.tile([128, CO, 128], BF16)
            wp = weights.tile([128, C], BF16)
            wp2 = weights.tile([D, C], BF16)
            nc.sync.dma_start(wq[:], wqT_ext[:])

            from collections import deque

            filler = deque()

            def load_x_tile(b, t):
                """x^T columns for 512 tokens: 8 chunk tiles of [128, 512]."""
                xs = []
                for co in range(CO):
                    xc = xpool.tile([128, 512], BF16, tag="xchunk", name="xc")
                    nc.sync.dma_start(
                        xc[:], xT_ext[b, t, co * 128:(co + 1) * 128, :]
                    )
                    xs.append(xc)
                return xs

            def qkv_chunk_units(b, t, xs, store):
                """One 512-token qkv tile -> QTc/KTc/VAc chunk tiles.

                Returns a list of single-PE-instruction closures."""
                units = []
                QTc = qkvp.tile([128, 512], BF16, tag="qt", name="qtc")
                KTc = qkvp.tile([128, 512], BF16, tag="kt", name="ktc")
                VAc = qkvp.tile([128, 4, 2 * (D + 1)], BF16, tag="vaug", name="vac")
                nc.vector.tensor_copy(VAc[:, :, D], onesb[:, 0:4])
                nc.vector.tensor_copy(VAc[:, :, 2 * D + 1], onesb[:, 0:4])
                store[t] = (QTc, KTc, VAc)

                def chain(w, writer):
                    hold = {}

                    def unit(co):
                        def f():
                            if co == 0:
                                hold["ps"] = ps_fill.tile(
                                    [128, 512], F32, tag="fill", name="fill_ps"
                                )
                            nc.tensor.matmul(
                                hold["ps"][:], lhsT=w[:, co, :], rhs=xs[co][:],
                                start=(co == 0), stop=(co == CO - 1),
                            )
                            if co == CO - 1:
                                writer(hold["ps"])
                        return f

                    return [unit(co) for co in range(CO)]

                vhold = {}

                def v_writer(ps):
                    vhold["vt"] = work.tile(
                        [128, 512], BF16, tag="vt", name="vt_stage"
                    )
                    nc.vector.tensor_copy(vhold["vt"][:], ps[:])

                units += chain(wq, lambda ps: nc.vector.tensor_copy(QTc[:], ps[:]))
                units += chain(wk, lambda ps: nc.vector.tensor_copy(KTc[:], ps[:]))
                units += chain(wv, v_writer)

                def transpose_unit(j):
                    def f():
                        pst = ps_fill.tile(
                            [128, 128], BF16, tag="fill", name="fill_t"
                        )
                        nc.tensor.transpose(
                            pst[:], vhold["vt"][:, j * 128:(j + 1) * 128], ident[:]
                        )
                        nc.vector.tensor_copy(
                            VAc[:, j, :]
                            .rearrange("p (h e) -> p h e", e=D + 1)[:, :, 0:D],
                            pst.rearrange("p (h d) -> p h d", d=D),
                        )
                    return f

                units += [transpose_unit(j) for j in range(4)]
                return units

            def queue_proj(on_stk, b, qt, onh1=None):
                def unit(sq, ot):
                    def f():
                        sqs = slice(sq * 128, (sq + 1) * 128)
                        qb = (b * N + qt * 512 + sq * 128) // 128
                        os_ = slice(ot * 512, (ot + 1) * 512)
                        yps = ps_fill.tile(
                            [128, 512], F32, tag="fill", name="fill_y"
                        )
                        if onh1 is None:
                            nc.tensor.matmul(
                                yps[:], lhsT=on_stk[:, sqs], rhs=wp[:, os_],
                                start=True, stop=True,
                            )
                        else:
                            # tail fast path: both heads from base-0 tiles,
                            # no partition-shift DMA on the critical path
                            nc.tensor.matmul(
                                yps[:], lhsT=on_stk[0:D, sqs], rhs=wp[0:D, os_],
                                start=True, stop=False,
                            )
                            nc.tensor.matmul(
                                yps[:], lhsT=onh1[:, sqs], rhs=wp2[:, os_],
                                start=False, stop=True,
                            )
                        ysb = ypool.tile([128, 512], BF16, tag="y", name="ysb")
                        nc.vector.tensor_copy(ysb[:], yps[:])
                        nc.sync.dma_start(out_ext[qb, ot], ysb[:])
                    return f

                for sq in range(4):
                    for ot in range(2):
                        filler.append(unit(sq, ot))

            PVLAG = 6

            def norm_and_queue_proj(O0, O1, b, qt, last=False):
                on_stk = onormp.tile([128, 512], BF16, tag="onstk", name="onstk")
                onh1 = None
                for h, O in ((0, O0), (1, O1)):
                    rec = small.tile([128, 512], F32, tag="rec", name="rec")
                    nc.vector.tensor_copy(rec[D:D + 1, :], O[D:D + 1, :])
                    rec0 = small.tile([128, 512], F32, tag="rec0", name="rec0")
                    nc.sync.dma_start(rec0[0:1, :], rec[D:D + 1, :])
                    sums = small.tile([D, 512], F32, tag="sums", name="sums")
                    nc.gpsimd.partition_broadcast(sums[:], rec0[0:1, :], channels=D)
                    bcs = small.tile([D, 512], F32, tag="bcs", name="bcs")
                    nc.vector.reciprocal_approx_fast(bcs[:], sums[:])
                    if h == 0:
                        nc.vector.tensor_mul(on_stk[0:D, :], O[0:D, :], bcs[:])
                    else:
                        onh = onormp.tile([D, 512], BF16, tag="onh", name="onh")
                        nc.vector.tensor_mul(onh[:], O[0:D, :], bcs[:])
                        if last:
                            onh1 = onh
                        else:
                            nc.sync.dma_start(on_stk[D:2 * D, :], onh[:])
                queue_proj(on_stk, b, qt, onh1)

            # ---- schedule: one global software pipeline ----
            store0, store1 = {}, {}
            x0 = [load_x_tile(0, 0)]
            nc.sync.dma_start(wk[:], wkT_ext[:])
            nc.sync.dma_start(wv[:], wvT_ext[:])
            x0 += [load_x_tile(0, t) for t in range(1, NT)]
            nc.sync.dma_start(wp[:], wpT_ext.rearrange("h p o -> (h p) o"))
            nc.sync.dma_start(wp2[:], wpT_ext[1])
            u0 = [qkv_chunk_units(0, t, x0[t], store0) for t in range(NT)]
            for u in u0[0][:16]:     # Q + K chains: needed by the first S
                u()
            x1 = [load_x_tile(1, t) for t in range(NT)]
            u1 = [qkv_chunk_units(1, t, x1[t], store1) for t in range(NT)]

            # qkv work in full-chain groups (atomic: they share one PSUM slot)
            qkv_groups = deque()

            def add_tile_groups(dl, ut):
                for c in range(3):
                    qkv_groups.append((dl, ut[8 * c:8 * c + 8], False))
                qkv_groups.append((dl, ut[24:28], False))

            qkv_groups.append((0, u0[0][16:24], False))
            qkv_groups.append((1, u0[0][24:28], False))
            for t in range(1, NT):
                add_tile_groups(4 * t, u0[t])
            for t in range(NT):
                add_tile_groups(52 + 4 * t, u1[t])
            n_groups_total = len(qkv_groups)
            pulled = [0]

            chain_open = [False]

            def pace_qkv(g):
                if qkv_groups and (
                    chain_open[0]
                    or qkv_groups[0][0] <= g + 3
                    or pulled[0] < (g * n_groups_total) // 70
                ):
                    dl, units, opens = qkv_groups.popleft()
                    for u in units:
                        u()
                    chain_open[0] = opens
                    pulled[0] += 1
                    return True
                return False

            stores = {0: store0, 1: store1}
            qts = [(0, q) for q in range(NT)] + [(1, q) for q in range(NT)]
            NQ = len(qts)
            Otiles = {}
            Ps = {}
            for g in range(NQ * NKT + PVLAG):
                # issue: S pair + exp for iteration g
                chain_pulled = False
                if g < NQ * NKT:
                    b, qt = qts[g // NKT]
                    kt = g % NKT
                    chain_pulled = pace_qkv(g)
                    QTc = stores[b][qt][0]
                    KTc = stores[b][kt // 4][1]
                    ks = slice((kt % 4) * 128, (kt % 4 + 1) * 128)
                    S = ps_s.tile([128, 2, 512], F32, tag="s", name="s")
                    nc.tensor.matmul(
                        S[:, 0, :], lhsT=KTc[0:D, ks], rhs=QTc[0:D, :],
                        start=True, stop=True, tile_position=(0, 0),
                    )
                    nc.tensor.matmul(
                        S[:, 1, :], lhsT=KTc[D:2 * D, ks], rhs=QTc[D:2 * D, :],
                        start=True, stop=True, tile_position=(64, 0),
                    )
                    P = work.tile([128, 2, 512], BF16, tag="p", name="p")
                    nc.scalar.activation(P[:], S[:], EXP)
                    Ps[g] = P
                # retire: PV pair for iteration g - PVLAG
                pv = g - PVLAG
                if pv >= 0:
                    bp, qtp = qts[pv // NKT]
                    ktp = pv % NKT
                    if ktp == 0:
                        Otiles[pv // NKT] = (
                            ps_o.tile([D + 1, 512], F32, tag="oacc", name="o0"),
                            ps_o.tile([D + 1, 512], F32, tag="oacc", name="o1"),
                        )
                    O0, O1 = Otiles[pv // NKT]
                    VAc = stores[bp][ktp // 4][2]
                    j = ktp % 4
                    P = Ps.pop(pv)
                    nc.tensor.matmul(
                        O0[:], lhsT=VAc[:, j, 0:D + 1], rhs=P[:, 0, :],
                        start=(ktp == 0), stop=(ktp == NKT - 1),
                    )
                    nc.tensor.matmul(
                        O1[:], lhsT=VAc[:, j, D + 1:2 * (D + 1)], rhs=P[:, 1, :],
                        start=(ktp == 0), stop=(ktp == NKT - 1),
                    )
                    if ktp == NKT - 1:
                        norm_and_queue_proj(
                            O0, O1, bp, qtp, last=(pv // NKT == NQ - 1)
                        )
                        del Otiles[pv // NKT]
                # drip proj fillers (not on chain iterations: proj shares the
                # single PSUM fill slot with open chains)
                if not chain_pulled:
                    for _ in range(2 if g >= 64 else 1):
                        if filler:
                            filler.popleft()()
            while filler or qkv_groups:
                if qkv_groups:
                    for u in qkv_groups.popleft()[1]:
                        u()
                elif filler:
                    filler.popleft()()
    nc.finalize()
    return nc


def _host_prep(x, W_qkv, W_proj):
    xT = np.ascontiguousarray(
        x.reshape(B, N // 512, 512, C).transpose(0, 1, 3, 2)
    ).astype(BFNP)
    in_maps = []
    for i in range(NCORES):
        hs = [HPC * i + j for j in range(HPC)]
        wq = np.concatenate([W_qkv[h * D:(h + 1) * D, :] for h in hs], 0)
        wk = np.concatenate([W_qkv[C + h * D:C + (h + 1) * D, :] for h in hs], 0)
        wv = np.concatenate([W_qkv[2 * C + h * D:2 * C + (h + 1) * D, :] for h in hs], 0)
        def chunkw(wT):
            return np.ascontiguousarray(
                wT.reshape(CO, 128, 128).transpose(1, 0, 2)
            ).astype(BFNP)

        wqT = chunkw((wq * SCALE).T)
        wkT = chunkw(wk.T)
        wvT = chunkw(wv.T)
        wpT = np.stack(
            [np.ascontiguousarray(W_proj[:, h * D:(h + 1) * D].T) for h in hs], 0
        ).astype(BFNP)
        in_maps.append({"xT": xT, "wqT": wqT, "wkT": wkT, "wvT": wvT, "wpT": wpT})
    return in_maps


def run(x, W_qkv, W_proj, b_proj, trace=False):
    if "nc" not in _NC_CACHE:
        _NC_CACHE["nc"] = build_nc()
    nc = _NC_CACHE["nc"]
    in_maps = _host_prep(np.asarray(x), np.asarray(W_qkv), np.asarray(W_proj))
    res = run_bass_kernel_spmd(
        nc, in_maps, core_ids=list(range(NCORES)), trace=trace
    )
    acc = np.zeros((BN, C), np.float64)
    for i in range(NCORES):
        o = res.results[i]["out"].astype(np.float64)   # [BN//128, 2, 128, 512]
        acc += o.transpose(0, 2, 1, 3).reshape(BN, C)
    y = (acc + np.asarray(b_proj).astype(np.float64)).astype(np.float32)
    return y.reshape(B, N, C), res


def kernel(x, W_qkv, W_proj, b_proj):
    y, _ = run(x, W_qkv, W_proj, b_proj, trace=False)
    return y


# revision 50
# speedup vs baseline: 1.0617x; 1.0036x over previous
"""Multi-head attention (B=2, N=2048, C=1024, H=16, D=64) on 8 TRN2 NeuronCores.

Sharding: tensor-parallel over heads. Core i owns heads (2i, 2i+1):
  - qkv weight columns for those heads (Q^T/K^T/V^T computed on device),
  - attention for 4 (batch, head) instances,
  - partial projection y_i = O_i @ W_proj[:, cols_i].T  (row-parallel proj).
Host gathers: y = sum_i y_i + b_proj.

Per-core pipeline (per batch):
  qkv:   Q^T,K^T [128(d,2 heads),2048] and V^T -> PE-transpose -> V_aug [n,130]
         (V columns + a ones column per head, so PV also yields softmax row-sums)
  attn:  per 512-query tile, loop over 16 key tiles:
         S^T[k,h,q] = K^T.T @ Q^T (bf16 in, f32 PSUM, the two heads run as
         packed row-group tiles), exp on ScalarE -> bf16,
         O~aug^T[65,q] += V_aug.T @ P~ (PSUM accumulation, row 64 = sum exp)
  norm:  rowsum row -> partition 0 (DMA) -> GpSimd broadcast ->
         fast reciprocal -> DVE mult (heads stacked for a K=128 projection)
  proj:  y[q,o] = sum_h O_norm^T_h.T @ WpT_h (PSUM accumulation over heads)

Batch 1's qkv work is emitted interleaved with batch 0's attention so the
TensorEngine stays dense (HAM stays at full clock) while ScalarE runs exp.
Matmul operands are bf16; softmax statistics, PSUM accumulation and the
final output stay float32.
"""
import sys
import types

import numpy as np

B = 2
N = 2048
C = 1024
H = 16
D = 64
SCALE = D ** -0.5
NCORES = 8
HPC = H // NCORES  # heads per core = 2
BN = B * N


def _install_ntff_shim():
    """The image's antenv lacks axon_hooks; provide it so trace=True works."""
    if "antenv.axon_hooks" in sys.modules:
        return
    mod = types.ModuleType("antenv.axon_hooks")
    mod._HOOK = None
    mod.set_axon_ntff_profile_hook = lambda h: setattr(mod, "_HOOK", h)
    mod.get_axon_ntff_profile_hook = lambda: mod._HOOK
    sys.modules["antenv.axon_hooks"] = mod
    if "/root/.axon_site" not in sys.path:
        sys.path.insert(0, "/root/.axon_site")
    try:
        from trn_agent_boot.trn_boot import _ntff_profile_via_ctypes

        mod.set_axon_ntff_profile_hook(
            _ntff_profile_via_ctypes("/opt/axon/libaxon_pjrt.so")
        )
    except Exception:
        pass


_install_ntff_shim()

import ml_dtypes  # noqa: E402

import concourse.bass as bass  # noqa: E402
import concourse.tile as tile  # noqa: E402
from concourse import bacc, mybir  # noqa: E402
from concourse.bass_utils import run_bass_kernel_spmd  # noqa: E402
from concourse.masks import make_identity  # noqa: E402

F32 = mybir.dt.float32
BF16 = mybir.dt.bfloat16
EXP = mybir.ActivationFunctionType.Exp
BFNP = ml_dtypes.bfloat16

NT = N // 512          # 512-token tiles per batch (4)
NKT = N // 128         # 128-token key tiles per batch (16)
CO = C // 128          # contraction chunks (8)

_NC_CACHE = {}


def build_nc():
    nc = bacc.Bacc(None, target_bir_lowering=False)

    xT_ext = nc.declare_dram_parameter("xT", [B, NT, C, 512], BF16, isOutput=False)
    wqT_ext = nc.declare_dram_parameter("wqT", [128, CO, 128], BF16, isOutput=False)
    wkT_ext = nc.declare_dram_parameter("wkT", [128, CO, 128], BF16, isOutput=False)
    wvT_ext = nc.declare_dram_parameter("wvT", [128, CO, 128], BF16, isOutput=False)
    wpT_ext = nc.declare_dram_parameter("wpT", [HPC, D, C], BF16, isOutput=False)
    out_ext = nc.declare_dram_parameter("out", [BN // 128, 2, 128, 512], BF16, isOutput=True)

    with tile.TileContext(nc) as tc:
        with (
            tc.tile_pool(name="consts", bufs=1) as consts,
            tc.tile_pool(name="weights", bufs=1) as weights,
            tc.tile_pool(name="xpool", bufs=8 * NT) as xpool,
            tc.tile_pool(name="qkvp", bufs=2 * NT) as qkvp,
            tc.tile_pool(name="work", bufs=8) as work,
            tc.tile_pool(name="small", bufs=3) as small,
            tc.tile_pool(name="onorm", bufs=6) as onormp,
            tc.tile_pool(name="ypool", bufs=4) as ypool,
            tc.tile_pool(name="ps_s", bufs=2, space="PSUM") as ps_s,
            tc.tile_pool(name="ps_o", bufs=3, space="PSUM") as ps_o,
            tc.tile_pool(name="ps_fill", bufs=1, space="PSUM") as ps_fill,
        ):
            wrm = consts.tile([128, 512], BF16)
            nc.vector.memset(wrm[:], 0.0)
            wps = ps_fill.tile([128, 512], F32, tag="fill", name="warm_ps")
            for _ in range(10):
                nc.tensor.matmul(
                    wps[:], lhsT=wrm[:, 0:128], rhs=wrm[:], start=True, stop=True
                )
            ident = consts.tile([128, 128], BF16)
            make_identity(nc, ident[:])
            onesb = consts.tile([128, 64], BF16)
            nc.vector.memset(onesb[:], 1.0)

            wq = weights.tile([128, CO, 128], BF16)
            wk = weights.tile([128, CO, 128], BF16)
            wv = weights.tile([128, CO, 128], BF16)
            wp = weights.tile([128, C], BF16)
            wp2 = weights.tile([D, C], BF16)
            nc.sync.dma_start(wq[:], wqT_ext[:])

            from collections import deque

            filler = deque()

            def load_x_tile(b, t):
                """x^T columns for 512 tokens: 8 chunk tiles of [128, 512]."""
                xs = []
                for co in range(CO):
                    xc = xpool.tile([128, 512], BF16, tag="xchunk", name="xc")
                    nc.sync.dma_start(
                        xc[:], xT_ext[b, t, co * 128:(co + 1) * 128, :]
                    )
                    xs.append(xc)
                return xs

            def qkv_chunk_units(b, t, xs, store):
                """One 512-token qkv tile -> QTc/KTc/VAc chunk tiles.

                Returns a list of single-PE-instruction closures."""
                units = []
                QTc = qkvp.tile([128, 512], BF16, tag="qt", name="qtc")
                KTc = qkvp.tile([128, 512], BF16, tag="kt", name="ktc")
                VAc = qkvp.tile([128, 4, 2 * (D + 1)], BF16, tag="vaug", name="vac")
                nc.vector.tensor_copy(VAc[:, :, D], onesb[:, 0:4])
                nc.vector.tensor_copy(VAc[:, :, 2 * D + 1], onesb[:, 0:4])
                store[t] = (QTc, KTc, VAc)

                def chain(w, writer):
                    hold = {}

                    def unit(co):
                        def f():
                            if co == 0:
                                hold["ps"] = ps_fill.tile(
                                    [128, 512], F32, tag="fill", name="fill_ps"
                                )
                            nc.tensor.matmul(
                                hold["ps"][:], lhsT=w[:, co, :], rhs=xs[co][:],
                                start=(co == 0), stop=(co == CO - 1),
                            )
                            if co == CO - 1:
                                writer(hold["ps"])
                        return f

                    return [unit(co) for co in range(CO)]

                vhold = {}

                def v_writer(ps):
                    vhold["vt"] = work.tile(
                        [128, 512], BF16, tag="vt", name="vt_stage"
                    )
                    nc.vector.tensor_copy(vhold["vt"][:], ps[:])

                units += chain(wq, lambda ps: nc.vector.tensor_copy(QTc[:], ps[:]))
                units += chain(wk, lambda ps: nc.vector.tensor_copy(KTc[:], ps[:]))
                units += chain(wv, v_writer)

                def transpose_unit(j):
                    def f():
                        pst = ps_fill.tile(
                            [128, 128], BF16, tag="fill", name="fill_t"
                        )
                        nc.tensor.transpose(
                            pst[:], vhold["vt"][:, j * 128:(j + 1) * 128], ident[:]
                        )
                        nc.vector.tensor_copy(
                            VAc[:, j, :]
                            .rearrange("p (h e) -> p h e", e=D + 1)[:, :, 0:D],
                            pst.rearrange("p (h d) -> p h d", d=D),
                        )
                    return f

                units += [transpose_unit(j) for j in range(4)]
                return units

            def queue_proj(on_stk, b, qt, onh1=None):
                def unit(sq, ot):
                    def f():
                        sqs = slice(sq * 128, (sq + 1) * 128)
                        qb = (b * N + qt * 512 + sq * 128) // 128
                        os_ = slice(ot * 512, (ot + 1) * 512)
                        yps = ps_fill.tile(
                            [128, 512], F32, tag="fill", name="fill_y"
                        )
                        if onh1 is None:
                            nc.tensor.matmul(
                                yps[:], lhsT=on_stk[:, sqs], rhs=wp[:, os_],
                                start=True, stop=True,
                            )
                        else:
                            # tail fast path: both heads from base-0 tiles,
                            # no partition-shift DMA on the critical path
                            nc.tensor.matmul(
                                yps[:], lhsT=on_stk[0:D, sqs], rhs=wp[0:D, os_],
                                start=True, stop=False,
                            )
                            nc.tensor.matmul(
                                yps[:], lhsT=onh1[:, sqs], rhs=wp2[:, os_],
                                start=False, stop=True,
                            )
                        ysb = ypool.tile([128, 512], BF16, tag="y", name="ysb")
                        nc.vector.tensor_copy(ysb[:], yps[:])
                        nc.sync.dma_start(out_ext[qb, ot], ysb[:])
                    return f

                for sq in range(4):
                    for ot in range(2):
                        filler.append(unit(sq, ot))

            PVLAG = 6

            def norm_and_queue_proj(O0, O1, b, qt, last=False):
                on_stk = onormp.tile([128, 512], BF16, tag="onstk", name="onstk")
                onh1 = None
                for h, O in ((0, O0), (1, O1)):
                    rec = small.tile([128, 512], F32, tag="rec", name="rec")
                    nc.vector.tensor_copy(rec[D:D + 1, :], O[D:D + 1, :])
                    rec0 = small.tile([128, 512], F32, tag="rec0", name="rec0")
                    nc.sync.dma_start(rec0[0:1, :], rec[D:D + 1, :])
                    sums = small.tile([D, 512], F32, tag="sums", name="sums")
                    nc.gpsimd.partition_broadcast(sums[:], rec0[0:1, :], channels=D)
                    bcs = small.tile([D, 512], F32, tag="bcs", name="bcs")
                    nc.vector.reciprocal_approx_fast(bcs[:], sums[:])
                    if h == 0:
                        nc.vector.tensor_mul(on_stk[0:D, :], O[0:D, :], bcs[:])
                    else:
                        onh = onormp.tile([D, 512], BF16, tag="onh", name="onh")
                        nc.vector.tensor_mul(onh[:], O[0:D, :], bcs[:])
                        if last:
                            onh1 = onh
                        else:
                            nc.sync.dma_start(on_stk[D:2 * D, :], onh[:])
                queue_proj(on_stk, b, qt, onh1)

            # ---- schedule: one global software pipeline ----
            store0, store1 = {}, {}
            x0 = [load_x_tile(0, 0)]
            nc.sync.dma_start(wk[:], wkT_ext[:])
            nc.sync.dma_start(wv[:], wvT_ext[:])
            x0 += [load_x_tile(0, t) for t in range(1, NT)]
            nc.sync.dma_start(wp[:], wpT_ext.rearrange("h p o -> (h p) o"))
            nc.sync.dma_start(wp2[:], wpT_ext[1])
            u0 = [qkv_chunk_units(0, t, x0[t], store0) for t in range(NT)]
            for u in u0[0][:16]:     # Q + K chains: needed by the first S
                u()
            x1 = [load_x_tile(1, t) for t in range(NT)]
            u1 = [qkv_chunk_units(1, t, x1[t], store1) for t in range(NT)]

            # qkv work in full-chain groups (atomic: they share one PSUM slot)
            qkv_groups = deque()

            def add_tile_groups(dl, ut):
                for c in range(3):
                    qkv_groups.append((dl, ut[8 * c:8 * c + 8], False))
                qkv_groups.append((dl, ut[24:28], False))

            qkv_groups.append((0, u0[0][16:24], False))
            qkv_groups.append((1, u0[0][24:28], False))
            for t in range(1, NT):
                add_tile_groups(4 * t, u0[t])
            for t in range(NT):
                add_tile_groups(52 + 4 * t, u1[t])
            n_groups_total = len(qkv_groups)
            pulled = [0]

            chain_open = [False]

            def pace_qkv(g):
                if qkv_groups and (
                    chain_open[0]
                    or qkv_groups[0][0] <= g + 3
                    or pulled[0] < (g * n_groups_total) // 70
                ):
                    dl, units, opens = qkv_groups.popleft()
                    for u in units:
                        u()
                    chain_open[0] = opens
                    pulled[0] += 1
                    return True
                return False

            stores = {0: store0, 1: store1}
            qts = [(0, q) for q in range(NT)] + [(1, q) for q in range(NT)]
            NQ = len(qts)
            Otiles = {}
            Ps = {}
            for g in range(NQ * NKT + PVLAG):
                # retire: PV pair for iteration g - PVLAG
                pv = g - PVLAG
                if pv >= 0:
                    bp, qtp = qts[pv // NKT]
                    ktp = pv % NKT
                    if ktp == 0:
                        Otiles[pv // NKT] = (
                            ps_o.tile([D + 1, 512], F32, tag="oacc", name="o0"),
                            ps_o.tile([D + 1, 512], F32, tag="oacc", name="o1"),
                        )
                    O0, O1 = Otiles[pv // NKT]
                    VAc = stores[bp][ktp // 4][2]
                    j = ktp % 4
                    P = Ps.pop(pv)
                    nc.tensor.matmul(
                        O0[:], lhsT=VAc[:, j, 0:D + 1], rhs=P[:, 0, :],
                        start=(ktp == 0), stop=(ktp == NKT - 1),
                    )
                    nc.tensor.matmul(
                        O1[:], lhsT=VAc[:, j, D + 1:2 * (D + 1)], rhs=P[:, 1, :],
                        start=(ktp == 0), stop=(ktp == NKT - 1),
                    )
                    if ktp == NKT - 1:
                        norm_and_queue_proj(
                            O0, O1, bp, qtp, last=(pv // NKT == NQ - 1)
                        )
                        del Otiles[pv // NKT]
                # issue: S pair + exp for iteration g
                chain_pulled = False
                if g < NQ * NKT:
                    b, qt = qts[g // NKT]
                    kt = g % NKT
                    chain_pulled = pace_qkv(g)
                    QTc = stores[b][qt][0]
                    KTc = stores[b][kt // 4][1]
                    ks = slice((kt % 4) * 128, (kt % 4 + 1) * 128)
                    S = ps_s.tile([128, 2, 512], F32, tag="s", name="s")
                    nc.tensor.matmul(
                        S[:, 0, :], lhsT=KTc[0:D, ks], rhs=QTc[0:D, :],
                        start=True, stop=True, tile_position=(0, 0),
                    )
                    nc.tensor.matmul(
                        S[:, 1, :], lhsT=KTc[D:2 * D, ks], rhs=QTc[D:2 * D, :],
                        start=True, stop=True, tile_position=(64, 0),
                    )
                    P = work.tile([128, 2, 512], BF16, tag="p", name="p")
                    nc.scalar.activation(P[:], S[:], EXP)
                    Ps[g] = P
                # drip proj fillers (not on chain iterations: proj shares the
                # single PSUM fill slot with open chains)
                if not chain_pulled:
                    for _ in range(2 if g >= 64 else 1):
                        if filler:
                            filler.popleft()()
            while filler or qkv_groups:
                if qkv_groups:
                    for u in qkv_groups.popleft()[1]:
                        u()
                elif filler:
                    filler.popleft()()
    nc.finalize()
    return nc


def _host_prep(x, W_qkv, W_proj):
    xT = np.ascontiguousarray(
        x.reshape(B, N // 512, 512, C).transpose(0, 1, 3, 2)
    ).astype(BFNP)
    in_maps = []
    for i in range(NCORES):
        hs = [HPC * i + j for j in range(HPC)]
        wq = np.concatenate([W_qkv[h * D:(h + 1) * D, :] for h in hs], 0)
        wk = np.concatenate([W_qkv[C + h * D:C + (h + 1) * D, :] for h in hs], 0)
        wv = np.concatenate([W_qkv[2 * C + h * D:2 * C + (h + 1) * D, :] for h in hs], 0)
        def chunkw(wT):
            return np.ascontiguousarray(
                wT.reshape(CO, 128, 128).transpose(1, 0, 2)
            ).astype(BFNP)

        wqT = chunkw((wq * SCALE).T)
        wkT = chunkw(wk.T)
        wvT = chunkw(wv.T)
        wpT = np.stack(
            [np.ascontiguousarray(W_proj[:, h * D:(h + 1) * D].T) for h in hs], 0
        ).astype(BFNP)
        in_maps.append({"xT": xT, "wqT": wqT, "wkT": wkT, "wvT": wvT, "wpT": wpT})
    return in_maps


def run(x, W_qkv, W_proj, b_proj, trace=False):
    if "nc" not in _NC_CACHE:
        _NC_CACHE["nc"] = build_nc()
    nc = _NC_CACHE["nc"]
    in_maps = _host_prep(np.asarray(x), np.asarray(W_qkv), np.asarray(W_proj))
    res = run_bass_kernel_spmd(
        nc, in_maps, core_ids=list(range(NCORES)), trace=trace
    )
    acc = np.zeros((BN, C), np.float64)
    for i in range(NCORES):
        o = res.results[i]["out"].astype(np.float64)   # [BN//128, 2, 128, 512]
        acc += o.transpose(0, 2, 1, 3).reshape(BN, C)
    y = (acc + np.asarray(b_proj).astype(np.float64)).astype(np.float32)
    return y.reshape(B, N, C), res


def kernel(x, W_qkv, W_proj, b_proj):
    y, _ = run(x, W_qkv, W_proj, b_proj, trace=False)
    return y


# revision 51
# speedup vs baseline: 1.0716x; 1.0093x over previous
"""Multi-head attention (B=2, N=2048, C=1024, H=16, D=64) on 8 TRN2 NeuronCores.

Sharding: tensor-parallel over heads. Core i owns heads (2i, 2i+1):
  - qkv weight columns for those heads (Q^T/K^T/V^T computed on device),
  - attention for 4 (batch, head) instances,
  - partial projection y_i = O_i @ W_proj[:, cols_i].T  (row-parallel proj).
Host gathers: y = sum_i y_i + b_proj.

Per-core pipeline (per batch):
  qkv:   Q^T,K^T [128(d,2 heads),2048] and V^T -> PE-transpose -> V_aug [n,130]
         (V columns + a ones column per head, so PV also yields softmax row-sums)
  attn:  per 512-query tile, loop over 16 key tiles:
         S^T[k,h,q] = K^T.T @ Q^T (bf16 in, f32 PSUM, the two heads run as
         packed row-group tiles), exp on ScalarE -> bf16,
         O~aug^T[65,q] += V_aug.T @ P~ (PSUM accumulation, row 64 = sum exp)
  norm:  rowsum row -> partition 0 (DMA) -> GpSimd broadcast ->
         fast reciprocal -> DVE mult (heads stacked for a K=128 projection)
  proj:  y[q,o] = sum_h O_norm^T_h.T @ WpT_h (PSUM accumulation over heads)

Batch 1's qkv work is emitted interleaved with batch 0's attention so the
TensorEngine stays dense (HAM stays at full clock) while ScalarE runs exp.
Matmul operands are bf16; softmax statistics, PSUM accumulation and the
final output stay float32.
"""
import sys
import types

import numpy as np

B = 2
N = 2048
C = 1024
H = 16
D = 64
SCALE = D ** -0.5
NCORES = 8
HPC = H // NCORES  # heads per core = 2
BN = B * N


def _install_ntff_shim():
    """The image's antenv lacks axon_hooks; provide it so trace=True works."""
    if "antenv.axon_hooks" in sys.modules:
        return
    mod = types.ModuleType("antenv.axon_hooks")
    mod._HOOK = None
    mod.set_axon_ntff_profile_hook = lambda h: setattr(mod, "_HOOK", h)
    mod.get_axon_ntff_profile_hook = lambda: mod._HOOK
    sys.modules["antenv.axon_hooks"] = mod
    if "/root/.axon_site" not in sys.path:
        sys.path.insert(0, "/root/.axon_site")
    try:
        from trn_agent_boot.trn_boot import _ntff_profile_via_ctypes

        mod.set_axon_ntff_profile_hook(
            _ntff_profile_via_ctypes("/opt/axon/libaxon_pjrt.so")
        )
    except Exception:
        pass


_install_ntff_shim()

import ml_dtypes  # noqa: E402

import concourse.bass as bass  # noqa: E402
import concourse.tile as tile  # noqa: E402
from concourse import bacc, mybir  # noqa: E402
from concourse.bass_utils import run_bass_kernel_spmd  # noqa: E402
from concourse.masks import make_identity  # noqa: E402

F32 = mybir.dt.float32
BF16 = mybir.dt.bfloat16
EXP = mybir.ActivationFunctionType.Exp
BFNP = ml_dtypes.bfloat16

NT = N // 512          # 512-token tiles per batch (4)
NKT = N // 128         # 128-token key tiles per batch (16)
CO = C // 128          # contraction chunks (8)

_NC_CACHE = {}


def build_nc():
    nc = bacc.Bacc(None, target_bir_lowering=False)

    xT_ext = nc.declare_dram_parameter("xT", [B, NT, C, 512], BF16, isOutput=False)
    wqT_ext = nc.declare_dram_parameter("wqT", [128, CO, 128], BF16, isOutput=False)
    wkT_ext = nc.declare_dram_parameter("wkT", [128, CO, 128], BF16, isOutput=False)
    wvT_ext = nc.declare_dram_parameter("wvT", [128, CO, 128], BF16, isOutput=False)
    wpT_ext = nc.declare_dram_parameter("wpT", [HPC, D, C], BF16, isOutput=False)
    out_ext = nc.declare_dram_parameter("out", [BN // 128, 2, 128, 512], BF16, isOutput=True)

    with tile.TileContext(nc) as tc:
        with (
            tc.tile_pool(name="consts", bufs=1) as consts,
            tc.tile_pool(name="weights", bufs=1) as weights,
            tc.tile_pool(name="xpool", bufs=8 * NT) as xpool,
            tc.tile_pool(name="qkvp", bufs=2 * NT) as qkvp,
            tc.tile_pool(name="work", bufs=8) as work,
            tc.tile_pool(name="small", bufs=3) as small,
            tc.tile_pool(name="onorm", bufs=6) as onormp,
            tc.tile_pool(name="ypool", bufs=4) as ypool,
            tc.tile_pool(name="ps_s", bufs=2, space="PSUM") as ps_s,
            tc.tile_pool(name="ps_o", bufs=3, space="PSUM") as ps_o,
            tc.tile_pool(name="ps_fill", bufs=1, space="PSUM") as ps_fill,
        ):
            wrm = consts.tile([128, 512], BF16)
            nc.vector.memset(wrm[:], 0.0)
            wps = ps_fill.tile([128, 512], F32, tag="fill", name="warm_ps")
            for _ in range(18):
                nc.tensor.matmul(
                    wps[:], lhsT=wrm[:, 0:128], rhs=wrm[:], start=True, stop=True
                )
            ident = consts.tile([128, 128], BF16)
            make_identity(nc, ident[:])
            onesb = consts.tile([128, 64], BF16)
            nc.vector.memset(onesb[:], 1.0)

            wq = weights.tile([128, CO, 128], BF16)
            wk = weights.tile([128, CO, 128], BF16)
            wv = weights.tile([128, CO, 128], BF16)
            wp = weights.tile([128, C], BF16)
            wp2 = weights.tile([D, C], BF16)
            nc.sync.dma_start(wq[:], wqT_ext[:])

            from collections import deque

            filler = deque()

            def load_x_tile(b, t):
                """x^T columns for 512 tokens: 8 chunk tiles of [128, 512]."""
                xs = []
                for co in range(CO):
                    xc = xpool.tile([128, 512], BF16, tag="xchunk", name="xc")
                    nc.sync.dma_start(
                        xc[:], xT_ext[b, t, co * 128:(co + 1) * 128, :]
                    )
                    xs.append(xc)
                return xs

            def qkv_chunk_units(b, t, xs, store):
                """One 512-token qkv tile -> QTc/KTc/VAc chunk tiles.

                Returns a list of single-PE-instruction closures."""
                units = []
                QTc = qkvp.tile([128, 512], BF16, tag="qt", name="qtc")
                KTc = qkvp.tile([128, 512], BF16, tag="kt", name="ktc")
                VAc = qkvp.tile([128, 4, 2 * (D + 1)], BF16, tag="vaug", name="vac")
                nc.vector.tensor_copy(VAc[:, :, D], onesb[:, 0:4])
                nc.vector.tensor_copy(VAc[:, :, 2 * D + 1], onesb[:, 0:4])
                store[t] = (QTc, KTc, VAc)

                def chain(w, writer):
                    hold = {}

                    def unit(co):
                        def f():
                            if co == 0:
                                hold["ps"] = ps_fill.tile(
                                    [128, 512], F32, tag="fill", name="fill_ps"
                                )
                            nc.tensor.matmul(
                                hold["ps"][:], lhsT=w[:, co, :], rhs=xs[co][:],
                                start=(co == 0), stop=(co == CO - 1),
                            )
                            if co == CO - 1:
                                writer(hold["ps"])
                        return f

                    return [unit(co) for co in range(CO)]

                vhold = {}

                def v_writer(ps):
                    vhold["vt"] = work.tile(
                        [128, 512], BF16, tag="vt", name="vt_stage"
                    )
                    nc.vector.tensor_copy(vhold["vt"][:], ps[:])

                units += chain(wq, lambda ps: nc.vector.tensor_copy(QTc[:], ps[:]))
                units += chain(wk, lambda ps: nc.vector.tensor_copy(KTc[:], ps[:]))
                units += chain(wv, v_writer)

                def transpose_unit(j):
                    def f():
                        pst = ps_fill.tile(
                            [128, 128], BF16, tag="fill", name="fill_t"
                        )
                        nc.tensor.transpose(
                            pst[:], vhold["vt"][:, j * 128:(j + 1) * 128], ident[:]
                        )
                        nc.vector.tensor_copy(
                            VAc[:, j, :]
                            .rearrange("p (h e) -> p h e", e=D + 1)[:, :, 0:D],
                            pst.rearrange("p (h d) -> p h d", d=D),
                        )
                    return f

                units += [transpose_unit(j) for j in range(4)]
                return units

            def queue_proj(on_stk, b, qt, onh1=None):
                def unit(sq, ot):
                    def f():
                        sqs = slice(sq * 128, (sq + 1) * 128)
                        qb = (b * N + qt * 512 + sq * 128) // 128
                        os_ = slice(ot * 512, (ot + 1) * 512)
                        yps = ps_fill.tile(
                            [128, 512], F32, tag="fill", name="fill_y"
                        )
                        if onh1 is None:
                            nc.tensor.matmul(
                                yps[:], lhsT=on_stk[:, sqs], rhs=wp[:, os_],
                                start=True, stop=True,
                            )
                        else:
                            # tail fast path: both heads from base-0 tiles,
                            # no partition-shift DMA on the critical path
                            nc.tensor.matmul(
                                yps[:], lhsT=on_stk[0:D, sqs], rhs=wp[0:D, os_],
                                start=True, stop=False,
                            )
                            nc.tensor.matmul(
                                yps[:], lhsT=onh1[:, sqs], rhs=wp2[:, os_],
                                start=False, stop=True,
                            )
                        ysb = ypool.tile([128, 512], BF16, tag="y", name="ysb")
                        nc.vector.tensor_copy(ysb[:], yps[:])
                        nc.sync.dma_start(out_ext[qb, ot], ysb[:])
                    return f

                for sq in range(4):
                    for ot in range(2):
                        filler.append(unit(sq, ot))

            PVLAG = 6

            def norm_and_queue_proj(O0, O1, b, qt, last=False):
                on_stk = onormp.tile([128, 512], BF16, tag="onstk", name="onstk")
                onh1 = None
                for h, O in ((0, O0), (1, O1)):
                    rec = small.tile([128, 512], F32, tag="rec", name="rec")
                    nc.vector.tensor_copy(rec[D:D + 1, :], O[D:D + 1, :])
                    rec0 = small.tile([128, 512], F32, tag="rec0", name="rec0")
                    nc.sync.dma_start(rec0[0:1, :], rec[D:D + 1, :])
                    sums = small.tile([D, 512], F32, tag="sums", name="sums")
                    nc.gpsimd.partition_broadcast(sums[:], rec0[0:1, :], channels=D)
                    bcs = small.tile([D, 512], F32, tag="bcs", name="bcs")
                    nc.vector.reciprocal_approx_fast(bcs[:], sums[:])
                    if h == 0:
                        nc.vector.tensor_mul(on_stk[0:D, :], O[0:D, :], bcs[:])
                    else:
                        onh = onormp.tile([D, 512], BF16, tag="onh", name="onh")
                        nc.vector.tensor_mul(onh[:], O[0:D, :], bcs[:])
                        if last:
                            onh1 = onh
                        else:
                            nc.sync.dma_start(on_stk[D:2 * D, :], onh[:])
                queue_proj(on_stk, b, qt, onh1)

            # ---- schedule: one global software pipeline ----
            store0, store1 = {}, {}
            x0 = [load_x_tile(0, 0)]
            nc.sync.dma_start(wk[:], wkT_ext[:])
            nc.sync.dma_start(wv[:], wvT_ext[:])
            x0 += [load_x_tile(0, t) for t in range(1, NT)]
            nc.sync.dma_start(wp[:], wpT_ext.rearrange("h p o -> (h p) o"))
            nc.sync.dma_start(wp2[:], wpT_ext[1])
            u0 = [qkv_chunk_units(0, t, x0[t], store0) for t in range(NT)]
            for u in u0[0][:16]:     # Q + K chains: needed by the first S
                u()
            x1 = [load_x_tile(1, t) for t in range(NT)]
            u1 = [qkv_chunk_units(1, t, x1[t], store1) for t in range(NT)]

            # qkv work in full-chain groups (atomic: they share one PSUM slot)
            qkv_groups = deque()

            def add_tile_groups(dl, ut):
                for c in range(3):
                    qkv_groups.append((dl, ut[8 * c:8 * c + 8], False))
                qkv_groups.append((dl, ut[24:28], False))

            qkv_groups.append((0, u0[0][16:24], False))
            qkv_groups.append((1, u0[0][24:28], False))
            for t in range(1, NT):
                add_tile_groups(4 * t, u0[t])
            for t in range(NT):
                add_tile_groups(52 + 4 * t, u1[t])
            n_groups_total = len(qkv_groups)
            pulled = [0]

            chain_open = [False]

            def pace_qkv(g):
                if qkv_groups and (
                    chain_open[0]
                    or qkv_groups[0][0] <= g + 3
                    or pulled[0] < (g * n_groups_total) // 70
                ):
                    dl, units, opens = qkv_groups.popleft()
                    for u in units:
                        u()
                    chain_open[0] = opens
                    pulled[0] += 1
                    return True
                return False

            stores = {0: store0, 1: store1}
            qts = [(0, q) for q in range(NT)] + [(1, q) for q in range(NT)]
            NQ = len(qts)
            Otiles = {}
            Ps = {}
            for g in range(NQ * NKT + PVLAG):
                # retire: PV pair for iteration g - PVLAG
                pv = g - PVLAG
                if pv >= 0:
                    bp, qtp = qts[pv // NKT]
                    ktp = pv % NKT
                    if ktp == 0:
                        Otiles[pv // NKT] = (
                            ps_o.tile([D + 1, 512], F32, tag="oacc", name="o0"),
                            ps_o.tile([D + 1, 512], F32, tag="oacc", name="o1"),
                        )
                    O0, O1 = Otiles[pv // NKT]
                    VAc = stores[bp][ktp // 4][2]
                    j = ktp % 4
                    P = Ps.pop(pv)
                    nc.tensor.matmul(
                        O0[:], lhsT=VAc[:, j, 0:D + 1], rhs=P[:, 0, :],
                        start=(ktp == 0), stop=(ktp == NKT - 1),
                    )
                    nc.tensor.matmul(
                        O1[:], lhsT=VAc[:, j, D + 1:2 * (D + 1)], rhs=P[:, 1, :],
                        start=(ktp == 0), stop=(ktp == NKT - 1),
                    )
                    if ktp == NKT - 1:
                        norm_and_queue_proj(
                            O0, O1, bp, qtp, last=(pv // NKT == NQ - 1)
                        )
                        del Otiles[pv // NKT]
                # issue: S pair + exp for iteration g
                chain_pulled = False
                if g < NQ * NKT:
                    b, qt = qts[g // NKT]
                    kt = g % NKT
                    chain_pulled = pace_qkv(g)
                    QTc = stores[b][qt][0]
                    KTc = stores[b][kt // 4][1]
                    ks = slice((kt % 4) * 128, (kt % 4 + 1) * 128)
                    S = ps_s.tile([128, 2, 512], F32, tag="s", name="s")
                    nc.tensor.matmul(
                        S[:, 0, :], lhsT=KTc[0:D, ks], rhs=QTc[0:D, :],
                        start=True, stop=True, tile_position=(0, 0),
                    )
                    nc.tensor.matmul(
                        S[:, 1, :], lhsT=KTc[D:2 * D, ks], rhs=QTc[D:2 * D, :],
                        start=True, stop=True, tile_position=(64, 0),
                    )
                    P = work.tile([128, 2, 512], BF16, tag="p", name="p")
                    nc.scalar.activation(P[:], S[:], EXP)
                    Ps[g] = P
                # drip proj fillers (not on chain iterations: proj shares the
                # single PSUM fill slot with open chains)
                if not chain_pulled:
                    for _ in range(2 if g >= 64 else 1):
                        if filler:
                            filler.popleft()()
            while filler or qkv_groups:
                if qkv_groups:
                    for u in qkv_groups.popleft()[1]:
                        u()
                elif filler:
                    filler.popleft()()
    nc.finalize()
    return nc


def _host_prep(x, W_qkv, W_proj):
    xT = np.ascontiguousarray(
        x.reshape(B, N // 512, 512, C).transpose(0, 1, 3, 2)
    ).astype(BFNP)
    in_maps = []
    for i in range(NCORES):
        hs = [HPC * i + j for j in range(HPC)]
        wq = np.concatenate([W_qkv[h * D:(h + 1) * D, :] for h in hs], 0)
        wk = np.concatenate([W_qkv[C + h * D:C + (h + 1) * D, :] for h in hs], 0)
        wv = np.concatenate([W_qkv[2 * C + h * D:2 * C + (h + 1) * D, :] for h in hs], 0)
        def chunkw(wT):
            return np.ascontiguousarray(
                wT.reshape(CO, 128, 128).transpose(1, 0, 2)
            ).astype(BFNP)

        wqT = chunkw((wq * SCALE).T)
        wkT = chunkw(wk.T)
        wvT = chunkw(wv.T)
        wpT = np.stack(
            [np.ascontiguousarray(W_proj[:, h * D:(h + 1) * D].T) for h in hs], 0
        ).astype(BFNP)
        in_maps.append({"xT": xT, "wqT": wqT, "wkT": wkT, "wvT": wvT, "wpT": wpT})
    return in_maps


def run(x, W_qkv, W_proj, b_proj, trace=False):
    if "nc" not in _NC_CACHE:
        _NC_CACHE["nc"] = build_nc()
    nc = _NC_CACHE["nc"]
    in_maps = _host_prep(np.asarray(x), np.asarray(W_qkv), np.asarray(W_proj))
    res = run_bass_kernel_spmd(
        nc, in_maps, core_ids=list(range(NCORES)), trace=trace
    )
    acc = np.zeros((BN, C), np.float64)
    for i in range(NCORES):
        o = res.results[i]["out"].astype(np.float64)   # [BN//128, 2, 128, 512]
        acc += o.transpose(0, 2, 1, 3).reshape(BN, C)
    y = (acc + np.asarray(b_proj).astype(np.float64)).astype(np.float32)
    return y.reshape(B, N, C), res


def kernel(x, W_qkv, W_proj, b_proj):
    y, _ = run(x, W_qkv, W_proj, b_proj, trace=False)
    return y
